# revision 26
# baseline (speedup 1.0000x reference)
"""Trainium2 Bass kernel for the AttentionLayer problem.

Contract: kernel(**inputs) takes FULL unsharded numpy inputs (as produced by
setup_inputs()) and returns the FULL [256, 2048] float32 output.

Strategy: pure data-parallel over the batch dim (32 batches per core, 8
cores).  All tensors are pre-packed on the host into the exact
channel-on-partition layouts the device kernel wants (this removes every
on-device transpose), then one SPMD Bass/Tile kernel runs on 8 NeuronCores.

Device dataflow (per core, everything "transposed": channels on partitions,
batch/rows in the free dim):
  sT,hT      [128,(16c,32b)]  <- host-packed s^T / h^T shards
  s_pT       = relu(s_proj_w^T-contraction)   via PE, psum [128,(16m,32b)]
  h_pT       = tanh(...)
  s_aT,h_aT  [128,(4m,32b)]
  per pair of batches (16 pairs):
    vt       [128,(16c,2b,196r)] <- host-packed v^T pair (3.2MB DMA)
    embed    = v^T @ v_att_w as 4x16 accumulated f32r matmuls -> psum[128,392]
    alpha_in = tanh(relu(embed + v_att_b) + h_a[b])           (DVE + ACT)
    scores   = alpha_w . alpha_in  via M=1 f32r matmuls -> psum[1,392]
    softmax per batch on partition 0; alpha broadcast to 128 partitions
    context  = sum_r alpha[r] * v^T[:, r] via fused DVE tensor_tensor_reduce
               (+ alpha[196]*s_p as the reduction's initial value)
  z^T        = tanh((ctx+h_p) @ ctx_w + b) via PE, written [128,(16m,32b)]
Host unpacks z^T back to [256, 2048].
"""

import numpy as np

D = 2048
H = 512
B = 256
R = 196
NC = 8          # cores
BL = B // NC    # 32 batches per core
NP = BL // 2    # 16 pairs per core
KC = D // 128   # 16 contraction chunks
MH = H // 128   # 4 H chunks

LAST_EXEC_NS = None
DEBUG = False

_compiled = None


def _build():
    import concourse.bass as bass
    import concourse.tile as tile
    from concourse import bacc, mybir
    from contextlib import ExitStack

    f32 = mybir.dt.float32
    f32r = mybir.dt.float32r
    bf16 = mybir.dt.bfloat16
    Act = mybir.ActivationFunctionType
    Alu = mybir.AluOpType
    ts = bass.ts

    nc = bacc.Bacc("TRN2", target_bir_lowering=False, debug=False, num_devices=NC)

    # ---- dram I/O (host-packed layouts) ----
    d_v = nc.dram_tensor("v", [NP, 128, KC * 2 * R], f32, kind="ExternalInput")
    d_sT = nc.dram_tensor("sT", [128, KC * BL], f32, kind="ExternalInput")
    d_hT = nc.dram_tensor("hT", [128, KC * BL], f32, kind="ExternalInput")
    d_vaw = nc.dram_tensor("vaw", [128, KC * H], f32, kind="ExternalInput")
    d_saw = nc.dram_tensor("saw", [4, 128, 4 * H], f32, kind="ExternalInput")
    d_haw = nc.dram_tensor("haw", [4, 128, 4 * H], f32, kind="ExternalInput")
    d_spw = nc.dram_tensor("spw", [KC, 128, D], f32, kind="ExternalInput")
    d_hpw = nc.dram_tensor("hpw", [KC, 128, D], f32, kind="ExternalInput")
    d_ctw = nc.dram_tensor("ctw", [KC, 128, D], f32, kind="ExternalInput")
    d_aw = nc.dram_tensor("aw", [128, MH], f32, kind="ExternalInput")
    d_vab = nc.dram_tensor("vab", [128, MH], f32, kind="ExternalInput")
    d_sab = nc.dram_tensor("sab", [128, MH], f32, kind="ExternalInput")
    d_hab = nc.dram_tensor("hab", [128, MH], f32, kind="ExternalInput")
    d_spb = nc.dram_tensor("spb", [128, KC], f32, kind="ExternalInput")
    d_hpb = nc.dram_tensor("hpb", [128, KC], f32, kind="ExternalInput")
    d_ctb = nc.dram_tensor("ctb", [128, KC], f32, kind="ExternalInput")
    d_z = nc.dram_tensor("zT", [128, KC * BL], f32, kind="ExternalOutput")
    if DEBUG:
        d_dbg_spT = nc.dram_tensor("dbg_spT", [128, KC * BL], f32, kind="ExternalOutput")
        d_dbg_haT = nc.dram_tensor("dbg_haT", [128, MH * BL], f32, kind="ExternalOutput")
        d_dbg_saT = nc.dram_tensor("dbg_saT", [128, MH * BL], f32, kind="ExternalOutput")
        d_dbg_es = nc.dram_tensor("dbg_es", [MH, 128, 2 * R], f32, kind="ExternalOutput")
        d_dbg_ain = nc.dram_tensor("dbg_ain", [MH, 128, 2 * R], f32, kind="ExternalOutput")
        d_dbg_sc = nc.dram_tensor("dbg_sc", [2, 197], f32, kind="ExternalOutput")
        d_dbg_al = nc.dram_tensor("dbg_al", [2, 197], f32, kind="ExternalOutput")
        d_dbg_abc = nc.dram_tensor("dbg_abc", [128, 197], f32, kind="ExternalOutput")
        d_dbg_ctx = nc.dram_tensor("dbg_ctx", [128, KC * BL], f32, kind="ExternalOutput")
        d_dbg_s197 = nc.dram_tensor("dbg_s197", [1, BL], f32, kind="ExternalOutput")
        d_dbg_psp = nc.dram_tensor("dbg_psp", [128, KC * BL], f32, kind="ExternalOutput")
        d_dbg_sTt = nc.dram_tensor("dbg_sTt", [128, KC * BL], f32, kind="ExternalOutput")

    W2 = 2 * R  # 392, moving-dim per pair (>=256 so f32r runs 1 cyc/row)

    with tile.TileContext(nc) as tc, ExitStack() as ctx:
        const = ctx.enter_context(tc.tile_pool(name="const", bufs=1))
        wpool = ctx.enter_context(tc.tile_pool(name="wpool", bufs=3))
        ctwp = ctx.enter_context(tc.tile_pool(name="ctwp", bufs=2))
        vpool = ctx.enter_context(tc.tile_pool(name="vpool", bufs=2))
        espool = ctx.enter_context(tc.tile_pool(name="espool", bufs=2))
        ainpool = ctx.enter_context(tc.tile_pool(name="ainpool", bufs=5))
        scrp = ctx.enter_context(tc.tile_pool(name="scrp", bufs=2))
        abcp = ctx.enter_context(tc.tile_pool(name="abcp", bufs=3))
        smp = ctx.enter_context(tc.tile_pool(name="smp", bufs=4))
        pe_ps = ctx.enter_context(tc.tile_pool(name="pe_ps", bufs=2, space="PSUM"))
        sc_ps = ctx.enter_context(tc.tile_pool(name="sc_ps", bufs=2, space="PSUM"))
        s197_ps = ctx.enter_context(tc.tile_pool(name="s197_ps", bufs=1, space="PSUM"))
        abc_ps = ctx.enter_context(tc.tile_pool(name="abc_ps", bufs=1, space="PSUM"))
        big_ps = ctx.enter_context(tc.tile_pool(name="big_ps", bufs=1, space="PSUM"))
        sm_ps = ctx.enter_context(tc.tile_pool(name="sm_ps", bufs=1, space="PSUM"))

        # ---- constants / small inputs ----
        vaw = const.tile([128, KC * H], bf16)
        nc.gpsimd.dma_start(vaw[:], d_vaw.ap())
        sT = const.tile([128, KC * BL], f32)
        nc.sync.dma_start(sT[:], d_sT.ap())
        hT = const.tile([128, KC * BL], f32)
        nc.sync.dma_start(hT[:], d_hT.ap())
        aw = const.tile([128, MH], bf16)
        nc.gpsimd.dma_start(aw[:], d_aw.ap())
        ones = const.tile([1, 128], f32)
        nc.gpsimd.memset(ones[:], 1.0)
        vab = const.tile([128, MH], f32)
        nc.sync.dma_start(vab[:], d_vab.ap())
        sab = const.tile([128, MH], f32)
        nc.sync.dma_start(sab[:], d_sab.ap())
        hab = const.tile([128, MH], f32)
        nc.sync.dma_start(hab[:], d_hab.ap())
        spb = const.tile([128, KC], f32)
        nc.sync.dma_start(spb[:], d_spb.ap())
        hpb = const.tile([128, KC], f32)
        nc.sync.dma_start(hpb[:], d_hpb.ap())
        ctb = const.tile([128, KC], f32)
        nc.sync.dma_start(ctb[:], d_ctb.ap())

        s_pT = const.tile([128, KC * BL], f32)
        h_pT = const.tile([128, KC * BL], f32)
        s_aT = const.tile([128, MH * BL], f32)
        h_aT = const.tile([128, MH * BL], f32)
        ctxT = const.tile([128, KC * BL], f32)
        zT = const.tile([128, KC * BL], f32)
        # 197th-region score for every local batch, on partition 0
        s197row = const.tile([1, BL], f32)

        # ---- prologue: s/h projections ----
        # s_p = relu(s @ s_proj_w + b) computed transposed: [128,(16m,32b)]
        for (wdram, outT, bias, fn) in (
            (d_spw, s_pT, spb, Act.Relu),
            (d_hpw, h_pT, hpb, Act.Tanh),
        ):
            psum = big_ps.tile([128, KC * BL], f32, tag="big")
            for k in range(KC):
                wt = wpool.tile([128, D], f32, tag="w")
                nc.sync.dma_start(wt[:], wdram[k])
                for m in range(KC):
                    nc.tensor.matmul(
                        psum[:, ts(m, BL)],
                        wt[:, ts(m, 128)],
                        (sT if outT is s_pT else hT)[:, ts(k, BL)],
                        start=(k == 0 and m == 0),
                        stop=(k == KC - 1 and m == KC - 1),
                        skip_group_check=True,
                    )
            if DEBUG and outT is s_pT:
                dbg_cp = scrp.tile([128, KC * BL], f32, tag="dbgp")
                nc.vector.tensor_copy(dbg_cp[:], psum[:])
                nc.sync.dma_start(d_dbg_psp.ap(), dbg_cp[:])
                nc.sync.dma_start(d_dbg_sTt.ap(), sT[:])
            for m in range(KC):
                nc.scalar.activation(
                    outT[:, ts(m, BL)], psum[:, ts(m, BL)], fn,
                    bias=bias[:, m : m + 1],
                )

        # s_a = s_p @ s_att_w + b ; h_a = h_p @ h_att_w + b   -> [128,(4m,32b)]
        for (wdram, inT, outT, bias) in (
            (d_saw, s_pT, s_aT, sab),
            (d_haw, h_pT, h_aT, hab),
        ):
            psum = sm_ps.tile([128, MH * BL], f32, tag="small")
            for j in range(4):
                wt = wpool.tile([128, D], f32, tag="w")
                nc.sync.dma_start(wt[:], wdram[j])
                for kk in range(4):
                    k = 4 * j + kk
                    for m in range(MH):
                        nc.tensor.matmul(
                            psum[:, ts(m, BL)],
                            wt[:, kk * H + m * 128 : kk * H + (m + 1) * 128],
                            inT[:, ts(k, BL)],
                            start=(k == 0 and m == 0),
                            stop=(k == KC - 1 and m == MH - 1),
                            skip_group_check=True,
                        )
            for m in range(MH):
                nc.scalar.activation(
                    outT[:, ts(m, BL)], psum[:, ts(m, BL)], Act.Identity,
                    bias=bias[:, m : m + 1],
                )

        # 197th region score for every batch: alpha_w . tanh(s_a + h_a)
        t197 = const.tile([128, MH * BL], bf16)
        tsum = scrp.tile([128, MH * BL], f32, tag="scr")
        nc.vector.tensor_add(tsum[:], s_aT[:], h_aT[:])
        nc.scalar.activation(t197[:], tsum[:], Act.Tanh)
        ps197 = s197_ps.tile([1, BL], f32, tag="s197")
        for m in range(MH):
            nc.tensor.matmul(
                ps197[:],
                aw[:, m : m + 1],
                t197[:, ts(m, BL)],
                start=(m == 0),
                stop=(m == MH - 1),
            )
        nc.scalar.activation(s197row[:], ps197[:], Act.Copy)
        if DEBUG:
            nc.sync.dma_start(d_dbg_spT.ap(), s_pT[:])
            nc.sync.dma_start(d_dbg_haT.ap(), h_aT[:])
            nc.sync.dma_start(d_dbg_saT.ap(), s_aT[:])
            nc.sync.dma_start(d_dbg_s197.ap(), s197row[:])

        # ---- main loop over batch pairs ----
        for p in range(NP):
            vt = vpool.tile([128, KC * W2], bf16, tag="vt")
            nc.gpsimd.dma_start(vt[:], d_v[p])

            ains = []
            for m in range(MH):
                pe = pe_ps.tile([128, W2], f32, tag="pe")
                for k in range(KC):
                    nc.tensor.matmul(
                        pe[:],
                        vaw[:, k * H + m * 128 : k * H + (m + 1) * 128],
                        vt[:, ts(k, W2)],
                        start=(k == 0),
                        stop=(k == KC - 1),
                    )
                es = espool.tile([128, W2], f32, tag="es")
                nc.vector.tensor_scalar(
                    out=es[:], in0=pe[:],
                    scalar1=vab[:, m : m + 1], scalar2=0.0,
                    op0=Alu.add, op1=Alu.max,
                )
                ain = ainpool.tile([128, W2], bf16, tag="ain")
                for j in range(2):
                    b = 2 * p + j
                    nc.scalar.activation(
                        ain[:, ts(j, R)], es[:, ts(j, R)], Act.Tanh,
                        bias=h_aT[:, m * BL + b : m * BL + b + 1],
                    )
                if DEBUG and p == 0:
                    nc.sync.dma_start(d_dbg_es[m], es[:])
                    nc.gpsimd.dma_start(d_dbg_ain[m], ain[:])
                ains.append(ain)

            pss = sc_ps.tile([1, W2], f32, tag="sc")
            for m in range(MH):
                nc.tensor.matmul(
                    pss[:],
                    aw[:, m : m + 1],
                    ains[m][:],
                    start=(m == 0),
                    stop=(m == MH - 1),
                )

            # both alphas of the pair assembled side by side on partition 0
            alpair = smp.tile([1, 2 * 197], f32, tag="alpair")
            for j in range(2):
                b = 2 * p + j
                # assemble this batch's 197 scores on partition 0
                sc_b = smp.tile([1, 197], f32, tag="sc_b")
                nc.scalar.activation(sc_b[0:1, 0:R], pss[0:1, ts(j, R)], Act.Copy)
                nc.scalar.activation(
                    sc_b[0:1, R : R + 1], s197row[0:1, b : b + 1], Act.Copy
                )
                # softmax over the 197 scores of batch b
                mx = smp.tile([1, 1], f32, tag="mx")
                nc.vector.reduce_max(mx[:], sc_b[:], axis=mybir.AxisListType.X)
                sub = smp.tile([1, 197], f32, tag="sub")
                nc.vector.tensor_scalar(
                    out=sub[:], in0=sc_b[:],
                    scalar1=mx[:, 0:1], scalar2=None,
                    op0=Alu.subtract,
                )
                exp_b = smp.tile([1, 197], f32, tag="exp_b")
                nc.scalar.activation(exp_b[:], sub[:], Act.Exp)
                sume = smp.tile([1, 1], f32, tag="sume")
                nc.vector.reduce_sum(sume[:], exp_b[:], axis=mybir.AxisListType.X)
                rec = smp.tile([1, 1], f32, tag="rec")
                nc.vector.reciprocal(rec[:], sume[:])
                nc.vector.tensor_scalar(
                    out=alpair[0:1, ts(j, 197)], in0=exp_b[:],
                    scalar1=rec[:, 0:1], scalar2=None,
                    op0=Alu.mult,
                )
                if DEBUG and p == 0:
                    nc.sync.dma_start(d_dbg_sc[j : j + 1], sc_b[:])
                    nc.sync.dma_start(d_dbg_al[j : j + 1], alpair[0:1, ts(j, 197)])
            # broadcast both alpha rows to all 128 partitions via ones-matmul
            ps_abc = abc_ps.tile([128, 2 * 197], f32, tag="abc")
            nc.tensor.matmul(ps_abc[:], ones[0:1, :], alpair[0:1, :],
                             start=True, stop=True)
            abc = abcp.tile([128, 2 * 197], bf16, tag="abc")
            nc.scalar.activation(abc[:], ps_abc[:], Act.Copy)
            if DEBUG and p == 0:
                nc.gpsimd.dma_start(d_dbg_abc.ap(), abc[:, 0:197])
            # context: ctx[:, (k,b)] = sum_r alpha[r] * v^T[:, r]  (+ alpha196*s_p)
            abc_v = abc[:].rearrange("q (j x) -> q j x", j=2)[:, :, 0:R]
            for k in range(KC):
                scr = scrp.tile([128, W2], bf16, tag="scr")
                nc.vector.tensor_tensor(
                    out=scr[:].rearrange("q (j x) -> q j x", j=2),
                    in0=vt[:, ts(k, W2)].rearrange("q (j x) -> q j x", j=2),
                    in1=abc_v,
                    op=Alu.mult,
                )
                for j in range(2):
                    nc.vector.reduce_sum(
                        ctxT[:, k * BL + 2 * p + j : k * BL + 2 * p + j + 1],
                        scr[:, ts(j, R)],
                        axis=mybir.AxisListType.X,
                    )
            # add the 197th-region (s_p) contribution for both batches
            for j in range(2):
                b = 2 * p + j
                wsp = smp.tile([128, KC], f32, tag="wsp")
                nc.vector.tensor_scalar(
                    out=wsp[:],
                    in0=s_pT[:].rearrange("q (c b) -> q c b", b=BL)[:, :, b],
                    scalar1=ps_abc[:, j * 197 + 196 : j * 197 + 197],
                    scalar2=None,
                    op0=Alu.mult,
                )
                ctx_b = ctxT[:].rearrange("q (c b) -> q c b", b=BL)[:, :, b]
                nc.vector.tensor_tensor(
                    out=ctx_b, in0=ctx_b, in1=wsp[:], op=Alu.add,
                )

        # ---- epilogue: z = tanh((ctx + h_p) @ ctx_w + b) ----
        if DEBUG:
            nc.sync.dma_start(d_dbg_ctx.ap(), ctxT[:])
        cph = ctxT  # in-place: ctx += h_p
        nc.vector.tensor_add(cph[:], ctxT[:], h_pT[:])
        psz = big_ps.tile([128, KC * BL], f32, tag="big")
        for k in range(KC):
            wt = ctwp.tile([128, D], f32, tag="ctw")
            nc.sync.dma_start(wt[:], d_ctw[k])
            for m in range(KC):
                nc.tensor.matmul(
                    psz[:, ts(m, BL)],
                    wt[:, ts(m, 128)],
                    cph[:, ts(k, BL)],
                    start=(k == 0 and m == 0),
                    stop=(k == KC - 1 and m == KC - 1),
                    skip_group_check=True,
                )
        for m in range(KC):
            nc.scalar.activation(
                zT[:, ts(m, BL)], psz[:, ts(m, BL)], Act.Tanh,
                bias=ctb[:, m : m + 1],
            )
        nc.sync.dma_start(d_z.ap(), zT[:])

    nc.compile()
    return nc


def _pack(v, s, h, v_att_w, v_att_b, s_proj_w, s_proj_b, s_att_w, s_att_b,
          h_proj_w, h_proj_b, h_att_w, h_att_b, alpha_w, alpha_b, ctx_w, ctx_b):
    f = np.float32
    v = np.asarray(v, f)
    # [core, pair, p, c, b2, r] -> [8, 16, 128, 6272]
    vp = v.reshape(NC, NP, 2, R, KC, 128).transpose(0, 1, 5, 4, 2, 3)
    vp = np.ascontiguousarray(vp).reshape(NC, NP, 128, KC * 2 * R)
    sT = np.ascontiguousarray(
        np.asarray(s, f).reshape(NC, BL, KC, 128).transpose(0, 3, 2, 1)
    ).reshape(NC, 128, KC * BL)
    hT = np.ascontiguousarray(
        np.asarray(h, f).reshape(NC, BL, KC, 128).transpose(0, 3, 2, 1)
    ).reshape(NC, 128, KC * BL)
    vaw = np.ascontiguousarray(
        np.asarray(v_att_w, f).reshape(KC, 128, H).transpose(1, 0, 2)
    ).reshape(128, KC * H)
    saw = np.ascontiguousarray(
        np.asarray(s_att_w, f).reshape(4, 4, 128, H).transpose(0, 2, 1, 3)
    ).reshape(4, 128, 4 * H)
    haw = np.ascontiguousarray(
        np.asarray(h_att_w, f).reshape(4, 4, 128, H).transpose(0, 2, 1, 3)
    ).reshape(4, 128, 4 * H)
    spw = np.ascontiguousarray(np.asarray(s_proj_w, f).reshape(KC, 128, D))
    hpw = np.ascontiguousarray(np.asarray(h_proj_w, f).reshape(KC, 128, D))
    ctw = np.ascontiguousarray(np.asarray(ctx_w, f).reshape(KC, 128, D))
    aw = np.ascontiguousarray(np.asarray(alpha_w, f).reshape(MH, 128).T)
    vab = np.ascontiguousarray(np.asarray(v_att_b, f).reshape(MH, 128).T)
    sab = np.ascontiguousarray(np.asarray(s_att_b, f).reshape(MH, 128).T)
    hab = np.ascontiguousarray(np.asarray(h_att_b, f).reshape(MH, 128).T)
    spb = np.ascontiguousarray(np.asarray(s_proj_b, f).reshape(KC, 128).T)
    hpb = np.ascontiguousarray(np.asarray(h_proj_b, f).reshape(KC, 128).T)
    ctb = np.ascontiguousarray(np.asarray(ctx_b, f).reshape(KC, 128).T)

    shared = dict(vaw=vaw, saw=saw, haw=haw, spw=spw, hpw=hpw, ctw=ctw,
                  aw=aw, vab=vab, sab=sab, hab=hab, spb=spb, hpb=hpb, ctb=ctb)
    in_maps = []
    for c in range(NC):
        m = dict(shared)
        m["v"] = vp[c]
        m["sT"] = sT[c]
        m["hT"] = hT[c]
        in_maps.append(m)
    return in_maps


def kernel(**inputs):
    global _compiled, LAST_EXEC_NS
    from concourse.bass_utils import run_bass_kernel_spmd

    in_maps = _pack(**inputs)
    if _compiled is None:
        _compiled = _build()
    res = run_bass_kernel_spmd(_compiled, in_maps, list(range(NC)))
    LAST_EXEC_NS = res.exec_time_ns
    if res.exec_time_ns is not None:
        print(f"HW exec time: {res.exec_time_ns} ns")
    out = np.empty((B, D), np.float32)
    for c in range(NC):
        zt = res.results[c]["zT"]  # [128, 16*32]
        out[c * BL : (c + 1) * BL] = (
            zt.reshape(128, KC, BL).transpose(2, 1, 0).reshape(BL, D)
        )
    return out


# revision 32
# speedup vs baseline: 1.6671x; 1.6671x over previous
"""Trainium2 Bass kernel for the AttentionLayer problem.

Contract: kernel(**inputs) takes FULL unsharded numpy inputs (as produced by
setup_inputs()) and returns the FULL [256, 2048] float32 output.

Strategy: pure data-parallel over the batch dim (32 batches per core, 8
cores).  All tensors are pre-packed on the host into the exact layouts the
device kernel wants (removes every on-device layout shuffle of v), then one
SPMD Bass/Tile kernel runs on 8 NeuronCores.

Device dataflow (per core):
  prologue (bf16 matmuls, weights as the MOVING operand so each k-chunk is
  one LDW + 4 N=512 matmuls):
    s_p = relu(s @ s_proj_w + b)  computed natural [32,2048] in psum,
          bias folded in as a ones-row extra contraction, then PE-transposed
          to s_pT [128,(16c,32b)]
    h_p likewise (tanh, kept f32 through the transpose)
    s_a,h_a  -> s_aT/h_aT [128,(4m,32b)] f32
    score of the 197th region for all 32 batches -> s197row [1,32]
  per pair of batches (16 pairs):
    vt    [128,(16c, 392+8pad)] bf16 <- f32->bf16 cast-DMA of host-packed v^T
    embed = v^T @ v_att_w: 4x16 accumulated bf16 matmuls -> psum[128,392]
    alpha_in = tanh(relu(embed + v_att_b) + h_a[b])       (DVE + ACT)
    scores   = alpha_w . alpha_in  via M=1 bf16 matmuls -> psum[1,392]
    softmax per batch on partition 0; alpha broadcast to 128 partitions via
    ones-matmul; context = per-chunk DVE mul + 3D reduce (+alpha196*s_p)
  epilogue (f32 for accuracy):
    z = tanh((ctx + h_p) @ ctx_w + b), natural [32,2048], bias via ones-row
Host unpacks the per-core z [32,2048] directly.
"""

import numpy as np

D = 2048
H = 512
B = 256
R = 196
NC = 8          # cores
BL = B // NC    # 32 batches per core
NP = BL // 2    # 16 pairs per core
KC = D // 128   # 16 contraction chunks
MH = H // 128   # 4 H chunks
W2 = 2 * R      # 392 moving rows per pair
WP = 400        # padded chunk width in vt (32B-aligned bf16 chunk starts)

LAST_EXEC_NS = None
DEBUG = False

_compiled = None


def _build():
    import concourse.bass as bass
    import concourse.tile as tile
    from concourse import bacc, mybir
    from contextlib import ExitStack

    f32 = mybir.dt.float32
    bf16 = mybir.dt.bfloat16
    Act = mybir.ActivationFunctionType
    Alu = mybir.AluOpType
    ts = bass.ts

    nc = bacc.Bacc("TRN2", target_bir_lowering=False, debug=False, num_devices=NC)

    # ---- dram I/O (host-packed layouts, all f32; bf16 via cast-DMA) ----
    d_v = nc.dram_tensor("v", [NP, 128, KC * W2], f32, kind="ExternalInput")
    d_sT = nc.dram_tensor("sT", [128, KC * BL], f32, kind="ExternalInput")
    d_hT = nc.dram_tensor("hT", [128, KC * BL], f32, kind="ExternalInput")
    d_vaw = nc.dram_tensor("vaw", [128, KC * H], f32, kind="ExternalInput")
    d_saw = nc.dram_tensor("saw", [KC, 128, H], f32, kind="ExternalInput")
    d_haw = nc.dram_tensor("haw", [KC, 128, H], f32, kind="ExternalInput")
    d_spw = nc.dram_tensor("spw", [KC, 128, D], f32, kind="ExternalInput")
    d_hpw = nc.dram_tensor("hpw", [KC, 128, D], f32, kind="ExternalInput")
    d_ctw = nc.dram_tensor("ctw", [KC, 128, D], f32, kind="ExternalInput")
    d_aw = nc.dram_tensor("aw", [128, MH], f32, kind="ExternalInput")
    d_vab = nc.dram_tensor("vab", [128, MH], f32, kind="ExternalInput")
    d_spbn = nc.dram_tensor("spbn", [1, D], f32, kind="ExternalInput")
    d_hpbn = nc.dram_tensor("hpbn", [1, D], f32, kind="ExternalInput")
    d_ctbn = nc.dram_tensor("ctbn", [1, D], f32, kind="ExternalInput")
    d_sabn = nc.dram_tensor("sabn", [1, H], f32, kind="ExternalInput")
    d_habn = nc.dram_tensor("habn", [1, H], f32, kind="ExternalInput")
    d_id = nc.dram_tensor("ident", [32, 32], f32, kind="ExternalInput")
    d_z = nc.dram_tensor("z", [BL, D], f32, kind="ExternalOutput")
    if DEBUG:
        d_dbg_spT = nc.dram_tensor("dbg_spT", [128, KC * BL], f32, kind="ExternalOutput")
        d_dbg_haT = nc.dram_tensor("dbg_haT", [128, MH * BL], f32, kind="ExternalOutput")
        d_dbg_saT = nc.dram_tensor("dbg_saT", [128, MH * BL], f32, kind="ExternalOutput")
        d_dbg_ain = nc.dram_tensor("dbg_ain", [MH, 128, W2], f32, kind="ExternalOutput")
        d_dbg_al = nc.dram_tensor("dbg_al", [1, W2], f32, kind="ExternalOutput")
        d_dbg_ctx = nc.dram_tensor("dbg_ctx", [128, KC * BL], f32, kind="ExternalOutput")
        d_dbg_s197 = nc.dram_tensor("dbg_s197", [1, BL], f32, kind="ExternalOutput")

    with tile.TileContext(nc) as tc, ExitStack() as ctx:
        const = ctx.enter_context(tc.tile_pool(name="const", bufs=1))
        wpool = ctx.enter_context(tc.tile_pool(name="wpool", bufs=3))
        ctwp = ctx.enter_context(tc.tile_pool(name="ctwp", bufs=4))
        vpool = ctx.enter_context(tc.tile_pool(name="vpool", bufs=3))
        espool = ctx.enter_context(tc.tile_pool(name="espool", bufs=2))
        ainpool = ctx.enter_context(tc.tile_pool(name="ainpool", bufs=5))
        scrp = ctx.enter_context(tc.tile_pool(name="scrp", bufs=3))
        abcp = ctx.enter_context(tc.tile_pool(name="abcp", bufs=2))
        natp = ctx.enter_context(tc.tile_pool(name="natp", bufs=1))
        smp = ctx.enter_context(tc.tile_pool(name="smp", bufs=4))
        pe_ps = ctx.enter_context(tc.tile_pool(name="pe_ps", bufs=2, space="PSUM"))
        sc_ps = ctx.enter_context(tc.tile_pool(name="sc_ps", bufs=1, space="PSUM"))
        nat_ps = ctx.enter_context(tc.tile_pool(name="nat_ps", bufs=1, space="PSUM"))
        tr_ps = ctx.enter_context(tc.tile_pool(name="tr_ps", bufs=1, space="PSUM"))

        # ---- constants / small inputs ----
        vaw = const.tile([128, KC * H], bf16)
        nc.gpsimd.dma_start(vaw[:], d_vaw.ap())
        sTb = const.tile([128, KC * BL], bf16)
        nc.gpsimd.dma_start(sTb[:], d_sT.ap())
        hTf = const.tile([128, KC * BL], f32)
        nc.sync.dma_start(hTf[:], d_hT.ap())
        aw = const.tile([128, MH], bf16)
        nc.gpsimd.dma_start(aw[:], d_aw.ap())
        vab = const.tile([128, MH], f32)
        nc.sync.dma_start(vab[:], d_vab.ap())
        spbn = const.tile([1, D], bf16)
        nc.gpsimd.dma_start(spbn[:], d_spbn.ap())
        hpbn = const.tile([1, D], f32)
        nc.sync.dma_start(hpbn[:], d_hpbn.ap())
        sabn = const.tile([1, H], bf16)
        nc.gpsimd.dma_start(sabn[:], d_sabn.ap())
        habn = const.tile([1, H], bf16)
        nc.gpsimd.dma_start(habn[:], d_habn.ap())
        ctbn = const.tile([1, D], f32)
        nc.sync.dma_start(ctbn[:], d_ctbn.ap())
        identb = const.tile([32, 32], bf16)
        nc.gpsimd.dma_start(identb[:], d_id.ap())
        identf = const.tile([32, 32], f32)
        nc.sync.dma_start(identf[:], d_id.ap())
        onesb = const.tile([1, 128], bf16)
        nc.gpsimd.memset(onesb[:], 1.0)
        onesf = const.tile([1, 128], f32)
        nc.gpsimd.memset(onesf[:], 1.0)

        s_pT = const.tile([128, KC * BL], bf16)
        h_pT = const.tile([128, KC * BL], f32)
        s_aT = const.tile([128, MH * BL], f32)
        h_aT = const.tile([128, MH * BL], f32)
        ctxT = const.tile([128, KC * BL], f32)
        t197 = const.tile([128, MH * BL], bf16)
        s197row = const.tile([1, BL], f32)

        # ---- prologue: s/h projections (bf16, weights moving) ----
        # natural psum [32, 2048]; stationary = sT/hT k-chunk [128, 32]
        for (wdram, xT, bnat, fn, pT) in (
            (d_spw, sTb, spbn, Act.Relu, s_pT),
            (d_hpw, hTf, hpbn, Act.Tanh, h_pT),
        ):
            wdt = bf16 if fn is Act.Relu else f32
            bones = onesb if fn is Act.Relu else onesf
            psn = nat_ps.tile([32, D], f32, tag="nat")
            for k in range(KC):
                if wdt is bf16:
                    wt = wpool.tile([128, D], bf16, tag="w")
                    nc.gpsimd.dma_start(wt[:], wdram[k])
                else:
                    wt = ctwp.tile([128, D], f32, tag="ctw")
                    nc.sync.dma_start(wt[:], wdram[k])
                for n in range(4):
                    nc.tensor.matmul(
                        psn[:, ts(n, 512)],
                        xT[:, ts(k, BL)],
                        wt[:, ts(n, 512)],
                        start=(k == 0),
                        stop=False,
                    )
            for n in range(4):  # bias via ones-row contraction
                nc.tensor.matmul(
                    psn[:, ts(n, 512)],
                    bones[0:1, 0:BL],
                    bnat[0:1, ts(n, 512)],
                    start=False,
                    stop=True,
                )
            tdt = bf16 if fn is Act.Relu else f32
            ident = identb if fn is Act.Relu else identf
            pnat = natp.tile([BL, D], tdt, tag="pnat" + fn.name)
            nc.scalar.activation(pnat[:], psn[:], fn)
            # transpose to [128,(16c,32b)]
            pst = tr_ps.tile([128, KC * BL], tdt, tag="tr")
            for c in range(KC):
                nc.tensor.matmul(
                    pst[:, ts(c, BL)], pnat[:, ts(c, 128)], ident[:],
                    is_transpose=True, skip_group_check=True,
                )
            nc.scalar.activation(pT[:], pst[:], Act.Copy)

        # s_a / h_a: stationary = s_pT/h_pT k-chunk, moving = att weights
        h_pTb = const.tile([128, KC * BL], bf16)
        nc.vector.tensor_copy(h_pTb[:], h_pT[:])
        for (wdram, inT, outT, bnat) in (
            (d_saw, s_pT, s_aT, sabn),
            (d_haw, h_pTb, h_aT, habn),
        ):
            psa = nat_ps.tile([32, H], f32, tag="nat")
            for k in range(KC):
                wt = wpool.tile([128, H], bf16, tag="watt")
                nc.gpsimd.dma_start(wt[:], wdram[k])
                nc.tensor.matmul(
                    psa[:], inT[:, ts(k, BL)], wt[:],
                    start=(k == 0), stop=False,
                )
            nc.tensor.matmul(
                psa[:], onesb[0:1, 0:BL], bnat[0:1, :],
                start=False, stop=True,
            )
            anat = natp.tile([BL, H], f32, tag="anat")
            nc.scalar.activation(anat[:], psa[:], Act.Copy)
            pst = tr_ps.tile([128, MH * BL], f32, tag="tr")
            for c in range(MH):
                nc.tensor.matmul(
                    pst[:, ts(c, BL)], anat[:, ts(c, 128)], identf[:],
                    is_transpose=True, skip_group_check=True,
                )
            nc.scalar.activation(outT[:], pst[:], Act.Copy)

        # 197th region score for every batch: alpha_w . tanh(s_a + h_a)
        tsum = scrp.tile([128, MH * BL], f32, tag="tsum")
        nc.vector.tensor_add(tsum[:], s_aT[:], h_aT[:])
        nc.scalar.activation(t197[:], tsum[:], Act.Tanh)
        ps197 = sc_ps.tile([1, BL], f32, tag="sc")
        for m in range(MH):
            nc.tensor.matmul(
                ps197[:],
                aw[:, m : m + 1],
                t197[:, ts(m, BL)],
                start=(m == 0),
                stop=(m == MH - 1),
            )
        nc.scalar.activation(s197row[:], ps197[:], Act.Copy)
        if DEBUG:
            nc.gpsimd.dma_start(d_dbg_spT.ap(), s_pT[:])
            nc.sync.dma_start(d_dbg_haT.ap(), h_aT[:])
            nc.sync.dma_start(d_dbg_saT.ap(), s_aT[:])
            nc.sync.dma_start(d_dbg_s197.ap(), s197row[:])

        # ---- main loop over batch pairs ----
        for p in range(NP):
            vt = vpool.tile([128, KC * WP], bf16, tag="vt")
            vt_v = vt[:].rearrange("q (c w) -> q c w", w=WP)
            nc.gpsimd.dma_start(vt_v[:, :, 0:W2], d_v[p])

            ains = []
            for m in range(MH):
                pe = pe_ps.tile([128, W2], f32, tag="pe")
                for k in range(KC):
                    nc.tensor.matmul(
                        pe[:],
                        vaw[:, k * H + m * 128 : k * H + (m + 1) * 128],
                        vt[:, k * WP : k * WP + W2],
                        start=(k == 0),
                        stop=(k == KC - 1),
                    )
                es = espool.tile([128, W2], f32, tag="es")
                nc.vector.tensor_scalar(
                    out=es[:], in0=pe[:],
                    scalar1=vab[:, m : m + 1], scalar2=0.0,
                    op0=Alu.add, op1=Alu.max,
                )
                ain = ainpool.tile([128, W2], bf16, tag="ain")
                for j in range(2):
                    b = 2 * p + j
                    nc.scalar.activation(
                        ain[:, ts(j, R)], es[:, ts(j, R)], Act.Tanh,
                        bias=h_aT[:, m * BL + b : m * BL + b + 1],
                    )
                if DEBUG and p == 0:
                    nc.gpsimd.dma_start(d_dbg_ain[m], ain[:])
                ains.append(ain)

            pss = sc_ps.tile([1, W2], f32, tag="sc")
            for m in range(MH):
                nc.tensor.matmul(
                    pss[:],
                    aw[:, m : m + 1],
                    ains[m][:],
                    start=(m == 0),
                    stop=(m == MH - 1),
                )

            # alpha rows for the pair, [0:196] per batch, bf16, plus a196
            alpair = smp.tile([1, W2], bf16, tag="alpair")
            al196 = smp.tile([1, 2], bf16, tag="al196")
            for j in range(2):
                b = 2 * p + j
                sc_b = smp.tile([1, 197], f32, tag="sc_b")
                nc.scalar.activation(sc_b[0:1, 0:R], pss[0:1, ts(j, R)], Act.Copy)
                nc.scalar.activation(
                    sc_b[0:1, R : R + 1], s197row[0:1, b : b + 1], Act.Copy
                )
                mx = smp.tile([1, 1], f32, tag="mx")
                nc.vector.reduce_max(mx[:], sc_b[:], axis=mybir.AxisListType.X)
                sub = smp.tile([1, 197], f32, tag="sub")
                nc.vector.tensor_scalar(
                    out=sub[:], in0=sc_b[:],
                    scalar1=mx[:, 0:1], scalar2=None,
                    op0=Alu.subtract,
                )
                exp_b = smp.tile([1, 197], f32, tag="exp_b")
                nc.scalar.activation(exp_b[:], sub[:], Act.Exp)
                sume = smp.tile([1, 1], f32, tag="sume")
                nc.vector.reduce_sum(sume[:], exp_b[:], axis=mybir.AxisListType.X)
                rec = smp.tile([1, 1], f32, tag="rec")
                nc.vector.reciprocal(rec[:], sume[:])
                nc.vector.tensor_scalar(
                    out=alpair[0:1, ts(j, R)], in0=exp_b[0:1, 0:R],
                    scalar1=rec[:, 0:1], scalar2=None,
                    op0=Alu.mult,
                )
                nc.vector.tensor_scalar(
                    out=al196[0:1, j : j + 1], in0=exp_b[0:1, R : R + 1],
                    scalar1=rec[:, 0:1], scalar2=None,
                    op0=Alu.mult,
                )
            # broadcast alphas to all 128 partitions via ones-matmul
            ps_abc = sc_ps.tile([128, W2], f32, tag="sc")
            nc.tensor.matmul(ps_abc[:], onesb[0:1, :], alpair[0:1, :],
                             start=True, stop=True)
            abc = abcp.tile([128, W2], bf16, tag="abc")
            nc.scalar.activation(abc[:], ps_abc[:], Act.Copy)
            ps_a196 = sc_ps.tile([128, 2], f32, tag="sc")
            nc.tensor.matmul(ps_a196[:], onesb[0:1, :], al196[0:1, :],
                             start=True, stop=True)
            if DEBUG and p == 0:
                nc.gpsimd.dma_start(d_dbg_al.ap(), abc[0:1, :])

            # context: ctx[:, (k, b)] = sum_r alpha[b, r] * v^T[:, (k, b, r)]
            for k in range(KC):
                scr = scrp.tile([128, W2], bf16, tag="scr")
                nc.vector.tensor_tensor(
                    out=scr[:], in0=vt[:, k * WP : k * WP + W2], in1=abc[:],
                    op=Alu.mult,
                )
                nc.vector.reduce_sum(
                    ctxT[:, k * BL + 2 * p : k * BL + 2 * p + 2],
                    scr[:].rearrange("q (j x) -> q j x", j=2),
                    axis=mybir.AxisListType.X,
                )
            # add the 197th-region (s_p) contribution for both batches
            for j in range(2):
                b = 2 * p + j
                wsp = smp.tile([128, KC], f32, tag="wsp")
                nc.vector.tensor_scalar(
                    out=wsp[:],
                    in0=s_pT[:].rearrange("q (c b) -> q c b", b=BL)[:, :, b],
                    scalar1=ps_a196[:, j : j + 1],
                    scalar2=None,
                    op0=Alu.mult,
                )
                ctx_b = ctxT[:].rearrange("q (c b) -> q c b", b=BL)[:, :, b]
                nc.vector.tensor_tensor(
                    out=ctx_b, in0=ctx_b, in1=wsp[:], op=Alu.add,
                )

        if DEBUG:
            nc.sync.dma_start(d_dbg_ctx.ap(), ctxT[:])

        # ---- epilogue: z = tanh((ctx + h_p) @ ctx_w + b), f32, natural ----
        cph = ctxT  # in-place
        nc.vector.tensor_add(cph[:], ctxT[:], h_pT[:])
        psz = nat_ps.tile([32, D], f32, tag="nat")
        for k in range(KC):
            wt = ctwp.tile([128, D], f32, tag="ctw")
            nc.sync.dma_start(wt[:], d_ctw[k])
            for n in range(4):
                nc.tensor.matmul(
                    psz[:, ts(n, 512)],
                    cph[:, ts(k, BL)],
                    wt[:, ts(n, 512)],
                    start=(k == 0),
                    stop=False,
                )
        for n in range(4):
            nc.tensor.matmul(
                psz[:, ts(n, 512)],
                onesf[0:1, 0:BL],
                ctbn[0:1, ts(n, 512)],
                start=False,
                stop=True,
            )
        znat = natp.tile([BL, D], f32, tag="znat")
        nc.scalar.activation(znat[:], psz[:], Act.Tanh)
        nc.sync.dma_start(d_z.ap(), znat[:])

    nc.compile()
    return nc


def _pack(v, s, h, v_att_w, v_att_b, s_proj_w, s_proj_b, s_att_w, s_att_b,
          h_proj_w, h_proj_b, h_att_w, h_att_b, alpha_w, alpha_b, ctx_w, ctx_b):
    f = np.float32
    v = np.asarray(v, f)
    # [core, pair, p, c, b2, r] -> [8, 16, 128, 6272]
    vp = v.reshape(NC, NP, 2, R, KC, 128).transpose(0, 1, 5, 4, 2, 3)
    vp = np.ascontiguousarray(vp).reshape(NC, NP, 128, KC * W2)
    sT = np.ascontiguousarray(
        np.asarray(s, f).reshape(NC, BL, KC, 128).transpose(0, 3, 2, 1)
    ).reshape(NC, 128, KC * BL)
    hT = np.ascontiguousarray(
        np.asarray(h, f).reshape(NC, BL, KC, 128).transpose(0, 3, 2, 1)
    ).reshape(NC, 128, KC * BL)
    vaw = np.ascontiguousarray(
        np.asarray(v_att_w, f).reshape(KC, 128, H).transpose(1, 0, 2)
    ).reshape(128, KC * H)
    saw = np.ascontiguousarray(np.asarray(s_att_w, f).reshape(KC, 128, H))
    haw = np.ascontiguousarray(np.asarray(h_att_w, f).reshape(KC, 128, H))
    spw = np.ascontiguousarray(np.asarray(s_proj_w, f).reshape(KC, 128, D))
    hpw = np.ascontiguousarray(np.asarray(h_proj_w, f).reshape(KC, 128, D))
    ctw = np.ascontiguousarray(np.asarray(ctx_w, f).reshape(KC, 128, D))
    aw = np.ascontiguousarray(np.asarray(alpha_w, f).reshape(MH, 128).T)
    vab = np.ascontiguousarray(np.asarray(v_att_b, f).reshape(MH, 128).T)

    shared = dict(
        vaw=vaw, saw=saw, haw=haw, spw=spw, hpw=hpw, ctw=ctw, aw=aw, vab=vab,
        spbn=np.asarray(s_proj_b, f).reshape(1, D),
        hpbn=np.asarray(h_proj_b, f).reshape(1, D),
        ctbn=np.asarray(ctx_b, f).reshape(1, D),
        sabn=np.asarray(s_att_b, f).reshape(1, H),
        habn=np.asarray(h_att_b, f).reshape(1, H),
        ident=np.eye(32, dtype=f),
    )
    in_maps = []
    for c in range(NC):
        m = dict(shared)
        m["v"] = vp[c]
        m["sT"] = sT[c]
        m["hT"] = hT[c]
        in_maps.append(m)
    return in_maps


def kernel(**inputs):
    global _compiled, LAST_EXEC_NS
    from concourse.bass_utils import run_bass_kernel_spmd

    in_maps = _pack(**inputs)
    if _compiled is None:
        _compiled = _build()
    res = run_bass_kernel_spmd(_compiled, in_maps, list(range(NC)))
    LAST_EXEC_NS = res.exec_time_ns
    if res.exec_time_ns is not None:
        print(f"HW exec time: {res.exec_time_ns} ns")
    out = np.empty((B, D), np.float32)
    for c in range(NC):
        out[c * BL : (c + 1) * BL] = res.results[c]["z"]
    return out
